# revision 15
# baseline (speedup 1.0000x reference)
"""DictionaryConv1D Trainium2 kernel (v2: fp16 + col-tile-paired PE).

reference:
  sparse = conv1d(x, dictionary, pad=4)        # [B, 64, L], 9-tap
  feat   = relu(w1 @ sparse + b1)              # [B, 256, L]
  out    = w2 @ feat + b2                      # [B, 64, L]

Strategy: data-parallel over batch (32 rows -> 4 per core on 8 cores).
All matmuls are fp16 M=64 in 128x64 col-tiled mode, issued as concurrent
pairs on PE col-tiles T0 (PSUM partitions 0-63) and T1 (64-127): a pair
costs ~the same as one M=128 matmul, so two 512-col L-tiles are processed
per "stage" at ~half the serial PE cost. The 9 conv taps run as 5
PSUM-accumulated K=128 matmuls (taps packed in pairs; tap 8 pairs with a
zero row). mm1 contracts the 64 atoms via half-zero lhsT (atoms of the even
tile on sp partitions 0-63, odd tile on 64-127). PSUM evacuation is the
bottleneck, so evacs are merged into [128]-wide ops: one copy moves both
tiles' sparse maps, each relu+bias evac spans two adjacent PSUM banks
(both tiles' features for one 128-channel chunk), and the final bias-add
moves both tiles' outputs at once. Output rides fp16 through SBUF/DRAM and
is cast to fp32 on the host.
"""
import sys

sys.path.insert(0, "/opt/trn_rl_repo")

import numpy as np
from contextlib import ExitStack

import concourse.bass as bass
import concourse.mybir as mybir
import concourse.tile as tile
from concourse.vector_clock import ScopedClock
from concourse.bass_utils import run_bass_kernel_spmd

B, C_IN, L = 32, 64, 8192
A, C_OUT, KTAPS = 64, 256, 9
NCORES = 8
BPC = B // NCORES          # batch rows per core
LPAD = L + 10              # 4 left pad + 5 right pad + 1 shift spare
NT = 512                   # L-tile columns (one PSUM bank)
NPAIRS = 5                 # taps 0-8 in pairs of 2 (pair 4 = tap 8 + zero)
NSTAGE = L // (2 * NT)     # stages per row: each stage = 2 L-tiles
f32 = mybir.dt.float32
f16 = mybir.dt.float16

MM_DT = f16
MM_NP = np.float16


class _SplitDrainTileContext(tile.TileContext):
    """This walrus build rejects instructions carrying >1 sync wait. Split
    every multi-wait instruction's extra waits onto same-engine nops placed
    immediately before it (same-engine program order makes this equivalent),
    and do the same for the Tile epilogue drain."""

    def _split_multi_waits(self):
        nc = self.nc
        for fn in nc.m.functions:
            for bb in fn.blocks:
                insts = list(bb.instructions)
                out = []
                changed = False
                for inst in insts:
                    si = inst.sync_info
                    if si is not None and len(si.on_wait) > 1:
                        ow = list(si.on_wait)
                        for w in ow[:-1]:
                            nop = mybir.InstNoOp(
                                name=nc.get_next_instruction_name(),
                                engine=inst.engine,
                                sync_info=mybir.SyncInfo(
                                    on_wait=[w], on_update=[]
                                ),
                                bass_nofuse=True,
                            )
                            nc.register_instruction(nop)
                            out.append(nop)
                        si.on_wait = ow[-1:]
                        inst.sync_info = si
                        changed = True
                    out.append(inst)
                if changed:
                    bb.instructions = out

    def _drain_and_barrier(self, tick_clock, wait_clock):
        probe = self.nc.sync.nop(nofuse=True)
        wait_clock.add_sem_waits(
            probe.ins, ScopedClock({None: tick_clock.global_clock})
        )
        si = probe.ins.sync_info
        if si is not None:
            ow = list(si.on_wait)
            if len(ow) > 1:
                si.on_wait = ow[:1]
                probe.ins.sync_info = si
                for w in ow[1:]:
                    extra = self.nc.sync.nop(nofuse=True)
                    extra.ins.sync_info = mybir.SyncInfo(on_wait=[w], on_update=[])
        self.nc.sync.drain()
        self._split_multi_waits()

        self.nc.all_engine_barrier()
        assert self.sems is not None
        popped = self.nc._tile_sem_poison_stack.pop()
        assert popped is self._sem_poison
        self.nc.clear_and_free_semaphores(list(self.sems.allocated().values()))
        self.nc.all_engine_barrier()


def _build_nc(reps=1, mm_dt=MM_DT, probe=()):
    # probe: timing-only ablations ("noe1" skip sparse evac, "noe3" skip
    # output evac+DMA). Numerics are wrong under probes; never set in kernel().
    nc = bass.Bass("TRN2", target_bir_lowering=False, debug=False,
                   num_devices=NCORES)
    xin = nc.dram_tensor("x", [BPC, C_IN, LPAD], mm_dt, kind="ExternalInput").ap()
    dl = nc.dram_tensor("dl", [128, NPAIRS * A], mm_dt, kind="ExternalInput").ap()
    w1lo = nc.dram_tensor("w1lo", [128, C_OUT], mm_dt, kind="ExternalInput").ap()
    w1hi = nc.dram_tensor("w1hi", [128, C_OUT], mm_dt, kind="ExternalInput").ap()
    w2ab = nc.dram_tensor("w2ab", [128, 128], mm_dt, kind="ExternalInput").ap()
    b1d = nc.dram_tensor("b1", [128, 2], f32, kind="ExternalInput").ap()
    b2d = nc.dram_tensor("b2", [128, 1], f32, kind="ExternalInput").ap()
    y = nc.dram_tensor("y", [BPC, C_IN, L], mm_dt, kind="ExternalOutput").ap()

    Relu = mybir.ActivationFunctionType.Relu
    Ident = mybir.ActivationFunctionType.Identity
    add = mybir.AluOpType.add
    amax = mybir.AluOpType.max

    with _SplitDrainTileContext(nc) as tc, ExitStack() as ctx:
        consts = ctx.enter_context(tc.tile_pool(name="consts", bufs=1))
        xrow_p = ctx.enter_context(tc.tile_pool(name="xrow", bufs=2))
        orow_p = ctx.enter_context(tc.tile_pool(name="orow", bufs=2))
        sp_p = ctx.enter_context(tc.tile_pool(name="sp", bufs=3))
        ftA_p = ctx.enter_context(tc.tile_pool(name="ftA", bufs=3))
        ftB_p = ctx.enter_context(tc.tile_pool(name="ftB", bufs=3))
        psA = ctx.enter_context(tc.tile_pool(name="psA", bufs=2, space="PSUM"))
        psB = ctx.enter_context(tc.tile_pool(name="psB", bufs=1, space="PSUM"))
        psC = ctx.enter_context(tc.tile_pool(name="psC", bufs=2, space="PSUM"))

        dl_sb = consts.tile([128, NPAIRS * A], mm_dt, tag="dl")
        nc.sync.dma_start(dl_sb[:], dl[:])
        w1lo_sb = consts.tile([128, C_OUT], mm_dt, tag="w1lo")
        w1hi_sb = consts.tile([128, C_OUT], mm_dt, tag="w1hi")
        w2ab_sb = consts.tile([128, 128], mm_dt, tag="w2ab")
        b1_sb = consts.tile([128, 2], f32, tag="b1")
        b2_sb = consts.tile([128, 1], f32, tag="b2")

        def load_tail_consts():
            nc.sync.dma_start(w1lo_sb[:], w1lo[:])
            nc.sync.dma_start(w1hi_sb[:], w1hi[:])
            nc.sync.dma_start(w2ab_sb[:], w2ab[:])
            nc.sync.dma_start(b1_sb[:], b1d[:])
            nc.sync.dma_start(b2_sb[:], b2d[:])

        # stage g = (row b, half-quarter m): covers L-tiles 2m (T0/lo) and
        # 2m+1 (T1/hi) of row b, i.e. columns [1024m, 1024m+1024).
        G = BPC * NSTAGE
        xts = {}
        orows = {}
        sps = {}
        fts = {}
        dbanks = {}

        def load_row_chunks(b, cuts):
            # partitions 0-63: x_pad; 64-127: x_pad shifted left by 1.
            # The DRAM-side AP reads the row twice, offset by one element,
            # so all 16 SBUF ports engage. Progressive chunks let stage-0
            # compute start before the whole row lands.
            xt = xts[b]
            for c0, c1 in zip(cuts[:-1], cuts[1:]):
                src = bass.AP(xin.tensor, b * C_IN * LPAD + c0,
                              [[1, 2], [LPAD, C_IN], [1, c1 - c0]])
                nc.sync.dma_start(xt[:, c0:c1], src)

        def init_row(b):
            xts[b] = xrow_p.tile([128, LPAD], mm_dt, tag="xt",
                                 name=f"xt{len(xts)}_{b}")
            orows[b] = orow_p.tile([128, L // 2], mm_dt, tag="orow",
                                   name=f"orow{len(orows)}_{b}")

        def s1(g):
            # conv pair: 10 interleaved M=64 matmuls -> C bank
            b, m = divmod(g, NSTAGE)
            if "noxdma" in probe:
                b, m = 0, 0
            if m == 0 and b not in xts:
                init_row(b)
                if "noxdma" in probe:
                    load_row_chunks(b, [0, 2 * NT + 10])
                else:
                    load_row_chunks(b, [0, 2 * NT + 10, 4 * NT + 10,
                                        8 * NT + 10, 12 * NT + 10, LPAD - 1])
            xt = xts[b]
            l0 = 2 * NT * m
            cps = psA.tile([128, NT], f32, tag="cps")
            for p in range(NPAIRS):
                nc.tensor.matmul(
                    cps[0:64, :],
                    dl_sb[:, A * p:A * (p + 1)],
                    xt[:, l0 + 2 * p:l0 + 2 * p + NT],
                    start=(p == 0), stop=(p == NPAIRS - 1),
                    skip_group_check=True,
                )
                nc.tensor.matmul(
                    cps[64:128, :],
                    dl_sb[:, A * p:A * (p + 1)],
                    xt[:, l0 + NT + 2 * p:l0 + NT + 2 * p + NT],
                    start=(p == 0), stop=(p == NPAIRS - 1),
                    skip_group_check=True,
                )
            sp = sp_p.tile([128, NT], mm_dt, tag="sp")
            if "noe1" not in probe:
                nc.vector.tensor_copy(sp[:], cps[:])
            sps[g] = sp

        def s2(g):
            # mm1: 8 M=64 matmuls, T0/T1 interleaved, into two 2-bank tiles
            sp = sps.pop(g)
            fA = psB.tile([128, 2 * NT], f32, tag="fA01")
            fB = psB.tile([128, 2 * NT], f32, tag="fB01")
            # channel chunk c -> partition half (c & 1); issue order
            # alternates T0/T1 so both col-tiles stream concurrently.
            # even tile's atoms on sp rows 0-63 -> w1lo; odd -> w1hi
            for ps, cpair in ((fA, (0, 1)), (fB, (2, 3))):
                for wsel, col0 in ((w1lo_sb, 0), (w1hi_sb, NT)):
                    for c in cpair:
                        p0 = 64 * (c & 1)
                        nc.tensor.matmul(ps[p0:p0 + 64, col0:col0 + NT],
                                         wsel[:, 64 * c:64 * c + 64], sp[:],
                                         start=True, stop=True,
                                         skip_group_check=True)
            ftA = ftA_p.tile([128, 2 * NT], mm_dt, tag="ftA")
            ftB = ftB_p.tile([128, 2 * NT], mm_dt, tag="ftB")
            # alternate engines by stage parity to balance DVE/ACT load
            if g & 1:
                nc.vector.tensor_scalar(ftA[:], fA[:], b1_sb[:, 0:1], 0.0,
                                        add, amax)
                nc.scalar.activation(ftB[:], fB[:], Relu, bias=b1_sb[:, 1:2])
            else:
                nc.scalar.activation(ftA[:], fA[:], Relu, bias=b1_sb[:, 0:1])
                nc.vector.tensor_scalar(ftB[:], fB[:], b1_sb[:, 1:2], 0.0,
                                        add, amax)
            fts[g] = (ftA, ftB)

        def s3(g):
            # mm2: 4 M=64 matmuls (2 accumulation steps per tile), T0/T1
            b, m = divmod(g, NSTAGE)
            ftA, ftB = fts.pop(g)
            ops = psC.tile([128, NT], f32, tag="ops")
            nc.tensor.matmul(ops[0:64, :], w2ab_sb[:, 0:64], ftA[:, 0:NT],
                             start=True, stop=False, skip_group_check=True)
            nc.tensor.matmul(ops[64:128, :], w2ab_sb[:, 0:64], ftA[:, NT:2 * NT],
                             start=True, stop=False, skip_group_check=True)
            nc.tensor.matmul(ops[0:64, :], w2ab_sb[:, 64:128], ftB[:, 0:NT],
                             start=False, stop=True, skip_group_check=True)
            nc.tensor.matmul(ops[64:128, :], w2ab_sb[:, 64:128], ftB[:, NT:2 * NT],
                             start=False, stop=True, skip_group_check=True)
            dbanks[g] = ops

        def e3(g):
            # bias-add both tiles' outputs into orow; fire DMA each half row
            b, m = divmod(g, NSTAGE)
            ops = dbanks.pop(g)
            if "noe3" in probe:
                return
            nc.scalar.activation(orows[b][:, NT * m:NT * (m + 1)], ops[:],
                                 Ident, bias=b2_sb[:, 0:1])
            if "nodma" in probe:
                return
            if m % 4 == 3:
                q = m // 4
                c0 = q * (L // 4)            # orow columns for this half row
                orow = orows[b]
                for par, off in ((0, 0), (64, NT)):
                    dst = bass.AP(y.tensor,
                                  b * C_IN * L + 2 * c0 + off,
                                  [[L, C_IN], [2 * NT, 4], [1, NT]])
                    nc.gpsimd.dma_start(
                        dst, orow[par:par + 64, c0:c0 + L // 4])

        # prologue: dl first, then row 0's first chunk, remaining consts,
        # rest of row 0; software pipeline with s1 two stages ahead
        init_row(0)
        load_row_chunks(0, [0, 2 * NT + 10])
        load_tail_consts()
        load_row_chunks(0, [2 * NT + 10, 4 * NT + 10, 8 * NT + 10,
                            12 * NT + 10, LPAD - 1])
        for r in range(reps):
            if r > 0:
                xts.clear()
                orows.clear()
            s1(0)
            s1(1)
            for g in range(G):
                s2(g)
                if g + 2 < G:
                    s1(g + 2)
                if g >= 1:
                    s3(g - 1)
                    e3(g - 1)
            s3(G - 1)
            e3(G - 1)

    return nc


_NC = None


def _get_nc():
    global _NC
    if _NC is None:
        _NC = _build_nc()
    return _NC


def _prep_inputs(x, dictionary, w1, b1, w2, b2, mm_np=MM_NP):
    x_pad = np.zeros((B, C_IN, LPAD), dtype=np.float32)
    x_pad[:, :, 4:4 + L] = x

    dlm = np.zeros((128, NPAIRS, A), dtype=np.float32)
    for p in range(NPAIRS):
        dlm[0:64, p, :] = dictionary[:, :, 2 * p].T
        if 2 * p + 1 < KTAPS:
            dlm[64:128, p, :] = dictionary[:, :, 2 * p + 1].T
    dlm = np.ascontiguousarray(dlm.reshape(128, NPAIRS * A))

    w1lo = np.zeros((128, C_OUT), dtype=np.float32)
    w1lo[0:64, :] = w1.T
    w1hi = np.zeros((128, C_OUT), dtype=np.float32)
    w1hi[64:128, :] = w1.T
    w2m = np.ascontiguousarray(w2.T.reshape(2, 128, C_IN).transpose(1, 0, 2)
                               .reshape(128, 2 * C_IN))
    b1m = np.ascontiguousarray(b1.reshape(2, 128).T)      # [128, 2]
    b2m = np.ascontiguousarray(np.concatenate([b2, b2]).reshape(128, 1))

    shared = {"dl": dlm.astype(mm_np), "w1lo": w1lo.astype(mm_np),
              "w1hi": w1hi.astype(mm_np), "w2ab": w2m.astype(mm_np),
              "b1": b1m, "b2": b2m}
    in_maps = []
    for c in range(NCORES):
        m = dict(shared)
        m["x"] = np.ascontiguousarray(x_pad[c * BPC:(c + 1) * BPC]).astype(mm_np)
        in_maps.append(m)
    return in_maps


def _unshard_out(res):
    # y is [BPC, 64, L/2 interleaved-pairs? no: [BPC, C_IN, L] f16 written
    # via the two de-interleaving DMAs, so it is already in natural layout.
    out = np.concatenate([res.results[c]["y"] for c in range(NCORES)], axis=0)
    return out.astype(np.float32)


def run(inputs, **kwargs):
    """Run on hardware; returns (out [B, C_IN, L], BassKernelResults)."""
    arrs = {k: np.asarray(v, dtype=np.float32) for k, v in inputs.items()}
    in_maps = _prep_inputs(arrs["x"], arrs["dictionary"], arrs["w1"],
                           arrs["b1"], arrs["w2"], arrs["b2"])
    res = run_bass_kernel_spmd(_get_nc(), in_maps,
                               core_ids=list(range(NCORES)), **kwargs)
    return _unshard_out(res), res


def kernel(**inputs):
    out, _ = run(inputs)
    return out


# revision 19
# speedup vs baseline: 1.8939x; 1.8939x over previous
"""DictionaryConv1D Trainium2 kernel (v2: fp16 + col-tile-paired PE).

reference:
  sparse = conv1d(x, dictionary, pad=4)        # [B, 64, L], 9-tap
  feat   = relu(w1 @ sparse + b1)              # [B, 256, L]
  out    = w2 @ feat + b2                      # [B, 64, L]

Strategy: data-parallel over batch (32 rows -> 4 per core on 8 cores).
All matmuls are fp16 M=64 in 128x64 col-tiled mode, issued as concurrent
pairs on PE col-tiles T0 (PSUM partitions 0-63) and T1 (64-127): a pair
costs ~the same as one M=128 matmul, so two 512-col L-tiles are processed
per "stage" at ~half the serial PE cost. The 9 conv taps run as 5
PSUM-accumulated K=128 matmuls (taps packed in pairs; tap 8 pairs with a
zero row). mm1 contracts the 64 atoms via half-zero lhsT (atoms of the even
tile on sp partitions 0-63, odd tile on 64-127). PSUM evacuation is the
bottleneck, so evacs are merged into [128]-wide ops: one copy moves both
tiles' sparse maps, each relu+bias evac spans two adjacent PSUM banks
(both tiles' features for one 128-channel chunk), and the final bias-add
moves both tiles' outputs at once. Output rides fp16 through SBUF/DRAM and
is cast to fp32 on the host.
"""
import sys

sys.path.insert(0, "/opt/trn_rl_repo")

import numpy as np
from contextlib import ExitStack

import concourse.bass as bass
import concourse.mybir as mybir
import concourse.tile as tile
from concourse.vector_clock import ScopedClock
from concourse.bass_utils import run_bass_kernel_spmd

B, C_IN, L = 32, 64, 8192
A, C_OUT, KTAPS = 64, 256, 9
NCORES = 8
BPC = B // NCORES          # batch rows per core
LPAD = L + 10              # 4 left pad + 5 right pad + 1 shift spare
NT = 512                   # L-tile columns (one PSUM bank)
NPAIRS = 5                 # taps 0-8 in pairs of 2 (pair 4 = tap 8 + zero)
NSTAGE = L // (2 * NT)     # stages per row: each stage = 2 L-tiles
f32 = mybir.dt.float32
f16 = mybir.dt.float16

MM_DT = f16
MM_NP = np.float16


class _SplitDrainTileContext(tile.TileContext):
    """This walrus build rejects instructions carrying >1 sync wait. Split
    every multi-wait instruction's extra waits onto same-engine nops placed
    immediately before it (same-engine program order makes this equivalent),
    and do the same for the Tile epilogue drain."""

    def _split_multi_waits(self):
        nc = self.nc
        for fn in nc.m.functions:
            for bb in fn.blocks:
                insts = list(bb.instructions)
                out = []
                changed = False
                for inst in insts:
                    si = inst.sync_info
                    if si is not None and len(si.on_wait) > 1:
                        ow = list(si.on_wait)
                        for w in ow[:-1]:
                            nop = mybir.InstNoOp(
                                name=nc.get_next_instruction_name(),
                                engine=inst.engine,
                                sync_info=mybir.SyncInfo(
                                    on_wait=[w], on_update=[]
                                ),
                                bass_nofuse=True,
                            )
                            nc.register_instruction(nop)
                            out.append(nop)
                        si.on_wait = ow[-1:]
                        inst.sync_info = si
                        changed = True
                    out.append(inst)
                if changed:
                    bb.instructions = out

    def _drain_and_barrier(self, tick_clock, wait_clock):
        probe = self.nc.sync.nop(nofuse=True)
        wait_clock.add_sem_waits(
            probe.ins, ScopedClock({None: tick_clock.global_clock})
        )
        si = probe.ins.sync_info
        if si is not None:
            ow = list(si.on_wait)
            if len(ow) > 1:
                si.on_wait = ow[:1]
                probe.ins.sync_info = si
                for w in ow[1:]:
                    extra = self.nc.sync.nop(nofuse=True)
                    extra.ins.sync_info = mybir.SyncInfo(on_wait=[w], on_update=[])
        self.nc.sync.drain()
        self._split_multi_waits()

        self.nc.all_engine_barrier()
        assert self.sems is not None
        popped = self.nc._tile_sem_poison_stack.pop()
        assert popped is self._sem_poison
        self.nc.clear_and_free_semaphores(list(self.sems.allocated().values()))
        self.nc.all_engine_barrier()


def _build_nc(reps=1, mm_dt=MM_DT, probe=()):
    # probe: timing-only ablations ("noe1" skip sparse evac, "noe3" skip
    # output evac+DMA). Numerics are wrong under probes; never set in kernel().
    nc = bass.Bass("TRN2", target_bir_lowering=False, debug=False,
                   num_devices=NCORES)
    xin = nc.dram_tensor("x", [BPC, C_IN, LPAD], mm_dt, kind="ExternalInput").ap()
    dl = nc.dram_tensor("dl", [128, NPAIRS * A], mm_dt, kind="ExternalInput").ap()
    w1lo = nc.dram_tensor("w1lo", [128, C_OUT], mm_dt, kind="ExternalInput").ap()
    w1hi = nc.dram_tensor("w1hi", [128, C_OUT], mm_dt, kind="ExternalInput").ap()
    w2ab = nc.dram_tensor("w2ab", [128, 128], mm_dt, kind="ExternalInput").ap()
    b1d = nc.dram_tensor("b1", [128, 2], f32, kind="ExternalInput").ap()
    b2d = nc.dram_tensor("b2", [128, 1], f32, kind="ExternalInput").ap()
    y = nc.dram_tensor("y", [BPC, C_IN, L], mm_dt, kind="ExternalOutput").ap()

    Relu = mybir.ActivationFunctionType.Relu
    Ident = mybir.ActivationFunctionType.Identity
    add = mybir.AluOpType.add
    amax = mybir.AluOpType.max

    with _SplitDrainTileContext(nc) as tc, ExitStack() as ctx:
        consts = ctx.enter_context(tc.tile_pool(name="consts", bufs=1))
        xrow_p = ctx.enter_context(tc.tile_pool(name="xrow", bufs=2))
        orow_p = ctx.enter_context(tc.tile_pool(name="orow", bufs=2))
        sp_p = ctx.enter_context(tc.tile_pool(name="sp", bufs=3))
        ftA_p = ctx.enter_context(tc.tile_pool(name="ftA", bufs=3))
        ftB_p = ctx.enter_context(tc.tile_pool(name="ftB", bufs=3))
        psA = ctx.enter_context(tc.tile_pool(name="psA", bufs=2, space="PSUM"))
        psB = ctx.enter_context(tc.tile_pool(name="psB", bufs=1, space="PSUM"))
        psC = ctx.enter_context(tc.tile_pool(name="psC", bufs=2, space="PSUM"))

        dl_sb = consts.tile([128, NPAIRS * A], mm_dt, tag="dl")
        nc.sync.dma_start(dl_sb[:], dl[:])
        w1lo_sb = consts.tile([128, C_OUT], mm_dt, tag="w1lo")
        w1hi_sb = consts.tile([128, C_OUT], mm_dt, tag="w1hi")
        w2ab_sb = consts.tile([128, 128], mm_dt, tag="w2ab")
        b1_sb = consts.tile([128, 2], f32, tag="b1")
        b2_sb = consts.tile([128, 1], f32, tag="b2")

        def load_tail_consts():
            nc.sync.dma_start(w1lo_sb[:], w1lo[:])
            nc.sync.dma_start(w1hi_sb[:], w1hi[:])
            nc.sync.dma_start(w2ab_sb[:], w2ab[:])
            nc.sync.dma_start(b1_sb[:], b1d[:])
            nc.sync.dma_start(b2_sb[:], b2d[:])

        # stage g = (row b, half-quarter m): covers L-tiles 2m (T0/lo) and
        # 2m+1 (T1/hi) of row b, i.e. columns [1024m, 1024m+1024).
        G = BPC * NSTAGE
        xts = {}
        orows = {}
        sps = {}
        fts = {}
        dbanks = {}

        def load_row_chunks(b, cuts):
            # partitions 0-63: x_pad; 64-127: x_pad shifted left by 1.
            # The DRAM-side AP reads the row twice, offset by one element,
            # so all 16 SBUF ports engage. Progressive chunks let stage-0
            # compute start before the whole row lands.
            xt = xts[b]
            for c0, c1 in zip(cuts[:-1], cuts[1:]):
                src = bass.AP(xin.tensor, b * C_IN * LPAD + c0,
                              [[1, 2], [LPAD, C_IN], [1, c1 - c0]])
                nc.sync.dma_start(xt[:, c0:c1], src)

        def init_row(b):
            xts[b] = xrow_p.tile([128, LPAD], mm_dt, tag="xt",
                                 name=f"xt{len(xts)}_{b}")
            orows[b] = orow_p.tile([128, L // 2], mm_dt, tag="orow",
                                   name=f"orow{len(orows)}_{b}")

        def s1(g):
            # conv pair: 10 interleaved M=64 matmuls -> C bank
            b, m = divmod(g, NSTAGE)
            if "noxdma" in probe:
                b, m = 0, 0
            if m == 0 and b not in xts:
                init_row(b)
                if "noxdma" in probe:
                    load_row_chunks(b, [0, 2 * NT + 10])
                else:
                    load_row_chunks(b, [0, 2 * NT + 10, 4 * NT + 10,
                                        8 * NT + 10, 12 * NT + 10, LPAD - 1])
            xt = xts[b]
            l0 = 2 * NT * m
            cps = psA.tile([128, NT], f32, tag="cps")
            for p in range(NPAIRS):
                nc.tensor.matmul(
                    cps[0:64, :],
                    dl_sb[:, A * p:A * (p + 1)],
                    xt[:, l0 + 2 * p:l0 + 2 * p + NT],
                    start=(p == 0), stop=(p == NPAIRS - 1),
                    skip_group_check=True,
                )
                nc.tensor.matmul(
                    cps[64:128, :],
                    dl_sb[:, A * p:A * (p + 1)],
                    xt[:, l0 + NT + 2 * p:l0 + NT + 2 * p + NT],
                    start=(p == 0), stop=(p == NPAIRS - 1),
                    skip_group_check=True,
                )
            sp = sp_p.tile([128, NT], mm_dt, tag="sp")
            if "noe1" not in probe:
                nc.vector.tensor_copy(sp[:], cps[:])
            sps[g] = sp

        def s2(g):
            # mm1: 8 M=64 matmuls, T0/T1 interleaved, into two 2-bank tiles
            sp = sps.pop(g)
            fA = psB.tile([128, 2 * NT], f32, tag="fA01")
            fB = psB.tile([128, 2 * NT], f32, tag="fB01")
            # channel chunk c -> partition half (c & 1); issue order
            # alternates T0/T1 so both col-tiles stream concurrently.
            # even tile's atoms on sp rows 0-63 -> w1lo; odd -> w1hi
            for ps, cpair in ((fA, (0, 1)), (fB, (2, 3))):
                for wsel, col0 in ((w1lo_sb, 0), (w1hi_sb, NT)):
                    for c in cpair:
                        p0 = 64 * (c & 1)
                        nc.tensor.matmul(ps[p0:p0 + 64, col0:col0 + NT],
                                         wsel[:, 64 * c:64 * c + 64], sp[:],
                                         start=True, stop=True,
                                         skip_group_check=True)
            ftA = ftA_p.tile([128, 2 * NT], mm_dt, tag="ftA")
            ftB = ftB_p.tile([128, 2 * NT], mm_dt, tag="ftB")
            # alternate engines by stage parity to balance DVE/ACT load
            if g & 1:
                nc.vector.tensor_scalar(ftA[:], fA[:], b1_sb[:, 0:1], 0.0,
                                        add, amax)
                nc.scalar.activation(ftB[:], fB[:], Relu, bias=b1_sb[:, 1:2])
            else:
                nc.scalar.activation(ftA[:], fA[:], Relu, bias=b1_sb[:, 0:1])
                nc.vector.tensor_scalar(ftB[:], fB[:], b1_sb[:, 1:2], 0.0,
                                        add, amax)
            fts[g] = (ftA, ftB)

        def s3(g):
            # mm2: 4 M=64 matmuls (2 accumulation steps per tile), T0/T1
            b, m = divmod(g, NSTAGE)
            ftA, ftB = fts.pop(g)
            ops = psC.tile([128, NT], f32, tag="ops")
            nc.tensor.matmul(ops[0:64, :], w2ab_sb[:, 0:64], ftA[:, 0:NT],
                             start=True, stop=False, skip_group_check=True)
            nc.tensor.matmul(ops[64:128, :], w2ab_sb[:, 0:64], ftA[:, NT:2 * NT],
                             start=True, stop=False, skip_group_check=True)
            nc.tensor.matmul(ops[0:64, :], w2ab_sb[:, 64:128], ftB[:, 0:NT],
                             start=False, stop=True, skip_group_check=True)
            nc.tensor.matmul(ops[64:128, :], w2ab_sb[:, 64:128], ftB[:, NT:2 * NT],
                             start=False, stop=True, skip_group_check=True)
            dbanks[g] = ops

        def e3(g):
            # bias-add both tiles' outputs into orow; fire DMA each quarter
            # row. Stays on ACT: DVE already carries e1 + one ft evac, so
            # {e2b, e3} on ACT vs {e1, e2a} on DVE is the balanced split.
            b, m = divmod(g, NSTAGE)
            ops = dbanks.pop(g)
            if "noe3" in probe:
                return
            nc.scalar.activation(orows[b][:, NT * m:NT * (m + 1)], ops[:],
                                 Ident, bias=b2_sb[:, 0:1])
            if "nodma" in probe:
                return
            if m % 2 == 1:
                q = m // 2
                c0 = q * (L // 8)            # orow columns for this 1/4 row
                orow = orows[b]
                for par, off in ((0, 0), (64, NT)):
                    dst = bass.AP(y.tensor,
                                  b * C_IN * L + 2 * c0 + off,
                                  [[L, C_IN], [2 * NT, 2], [1, NT]])
                    nc.gpsimd.dma_start(
                        dst, orow[par:par + 64, c0:c0 + L // 8])

        # prologue: dl first, then row 0's first chunk, remaining consts,
        # rest of row 0; software pipeline with s1 two stages ahead
        init_row(0)
        load_row_chunks(0, [0, 2 * NT + 10])
        load_tail_consts()
        load_row_chunks(0, [2 * NT + 10, 4 * NT + 10, 8 * NT + 10,
                            12 * NT + 10, LPAD - 1])
        # HAM warmup: a few throwaway matmuls on dl (first DMA to land) so
        # the PE's busy window opens during the x-chunk DMA wait; the bank
        # is a psC tile that s3 later start-overwrites.
        warm = psC.tile([128, NT], f32, tag="ops")
        for _ in range(5):
            nc.tensor.matmul(warm[0:64, 0:NPAIRS * A], dl_sb[:, 0:64], dl_sb[:],
                             start=True, stop=True, skip_group_check=True)
        for r in range(reps):
            if r > 0:
                xts.clear()
                orows.clear()
            s1(0)
            s1(1)
            for g in range(G):
                s2(g)
                if g + 2 < G:
                    s1(g + 2)
                if g >= 1:
                    s3(g - 1)
                    e3(g - 1)
            s3(G - 1)
            e3(G - 1)

    return nc


_NC = None


def _get_nc():
    global _NC
    if _NC is None:
        _NC = _build_nc()
    return _NC


def _prep_inputs(x, dictionary, w1, b1, w2, b2, mm_np=MM_NP):
    x_pad = np.zeros((B, C_IN, LPAD), dtype=np.float32)
    x_pad[:, :, 4:4 + L] = x

    dlm = np.zeros((128, NPAIRS, A), dtype=np.float32)
    for p in range(NPAIRS):
        dlm[0:64, p, :] = dictionary[:, :, 2 * p].T
        if 2 * p + 1 < KTAPS:
            dlm[64:128, p, :] = dictionary[:, :, 2 * p + 1].T
    dlm = np.ascontiguousarray(dlm.reshape(128, NPAIRS * A))

    w1lo = np.zeros((128, C_OUT), dtype=np.float32)
    w1lo[0:64, :] = w1.T
    w1hi = np.zeros((128, C_OUT), dtype=np.float32)
    w1hi[64:128, :] = w1.T
    w2m = np.ascontiguousarray(w2.T.reshape(2, 128, C_IN).transpose(1, 0, 2)
                               .reshape(128, 2 * C_IN))
    b1m = np.ascontiguousarray(b1.reshape(2, 128).T)      # [128, 2]
    b2m = np.ascontiguousarray(np.concatenate([b2, b2]).reshape(128, 1))

    shared = {"dl": dlm.astype(mm_np), "w1lo": w1lo.astype(mm_np),
              "w1hi": w1hi.astype(mm_np), "w2ab": w2m.astype(mm_np),
              "b1": b1m, "b2": b2m}
    in_maps = []
    for c in range(NCORES):
        m = dict(shared)
        m["x"] = np.ascontiguousarray(x_pad[c * BPC:(c + 1) * BPC]).astype(mm_np)
        in_maps.append(m)
    return in_maps


def _unshard_out(res):
    # y is [BPC, 64, L/2 interleaved-pairs? no: [BPC, C_IN, L] f16 written
    # via the two de-interleaving DMAs, so it is already in natural layout.
    out = np.concatenate([res.results[c]["y"] for c in range(NCORES)], axis=0)
    return out.astype(np.float32)


def run(inputs, **kwargs):
    """Run on hardware; returns (out [B, C_IN, L], BassKernelResults)."""
    arrs = {k: np.asarray(v, dtype=np.float32) for k, v in inputs.items()}
    in_maps = _prep_inputs(arrs["x"], arrs["dictionary"], arrs["w1"],
                           arrs["b1"], arrs["w2"], arrs["b2"])
    res = run_bass_kernel_spmd(_get_nc(), in_maps,
                               core_ids=list(range(NCORES)), **kwargs)
    return _unshard_out(res), res


def kernel(**inputs):
    out, _ = run(inputs)
    return out
